# revision 16
# baseline (speedup 1.0000x reference)
"""Kent-distribution pairwise KLD loss kernel for Trainium2 (8 NeuronCores).

The [N, M] pairwise KLD matrix factors exactly as a rank-11 product
U @ V^T; N (pred rows) is sharded across the 8 cores.  The pairwise
part runs as ONE bf16 matmul per 512-column chunk (1 cyc/col vs fp32's
2 passes x 4 cyc/col) with contraction K=30:

  V30 = [Vh(11); Vl(4);     Vh(11); Vl(4)]
  U30 = [Uh(11); Uh(f1..4); Ul(11); Ul(f1..4)]
  sum_k U30[k] V30[k] = (Uh+Ul) . Vh + (Uh+Ul)_{f1..4} . Vl_{f1..4}

i.e. split-float bf16 hi+lo (exact to ~2^-17) for U and for the four
large V features (c_b, -k*gamma_b1); the six beta features are bf16-
only, contributing < 3e-3 rel error (gate is 2e-2; measured 3e-4).

Algebraic reductions vs the reference:
 - l1 = (k^2-k-s)/D, l2 = ((k-1)k^2 - ks - s/2)/D^2, D = k^2-s,
   s = 4b^2 (exact ratios; exp table never needed, exp(-EPS) dropped).
 - |gamma1|^2 == 1 exactly => kappa_a.Ex_a = k*l1.
 - l2 * sum(dVdiag) == 0 (unit gammas) => l2 dropped from UF[5:8].
 - LN_2PI cancels between c_b and -c_a => dropped from both.
 - G3 := -gamma3 (only quadratic uses; sign-insensitive).
 - Sin HW domain is [-pi,pi]: cos(x) = sin(pi/2 - |x|).

Trace-driven scheduling (engine queues are strictly in-order):
 - Input DMAs issue first on both HWDGE rings; a dummy Sin loads the
   trig ACT table during the ~4us DMA latency.  ACT then runs sin,
   cos, [ln-table], ln, and afterwards only table-free COPYs (each
   Sin<->Ln switch would reload a 1.28us table).
 - Elementwise chain is packed via strided/negative-stride APs over
   one workspace tile W (slot axis x 18 group columns); U-side scalar
   chain on DVE via fused scalar_tensor_tensor (Pool lacks it), gamma
   products split across DVE/Pool, V features in two 8-group halves.
 - VT stays group-major (col (j, p) <-> target m = 16p + j, fixed by
   the per-partition-contiguous input DMA); the matmul moving AP
   [30, 32, 16] emits columns in m order.  The ~2x strided-read cost
   on the PE measured cheaper than every alternative scatter carrier.
 - PSUM->SBUF out copies on DVE only (ACT's copy+DMA-issue could not
   keep the 860ns/block matmul pace); DMAs alternate both rings in
   256-col halves, the final block drains in 128-col quarters.
"""

import sys

import numpy as np

sys.path.insert(0, "/opt/trn_rl_repo")

import concourse.bass as bass  # noqa: E402,F401
import concourse.mybir as mybir  # noqa: E402
import concourse.tile as tile  # noqa: E402
from concourse import bacc  # noqa: E402
from concourse.masks import make_identity  # noqa: E402

F32 = mybir.dt.float32
BF16 = mybir.dt.bfloat16
AF = mybir.ActivationFunctionType
ALU = mybir.AluOpType

N = 2048
M = 2048
NCORES = 8
NS = N // NCORES
K = 11
KL = 4  # features keeping a bf16 lo part (f1..f4: c_b, -k*g_b1)
KB = K + KL  # 15: one [Vh(11); Vl(4)] block
K2 = 2 * KB  # 30: contraction size
GP = NS // 128  # 2
GT = M // 128  # 16
G = GP + GT  # 18

PI = float(np.pi)
EPS = 1e-6


def _body(tc, pred, targ, out):
    nc = tc.nc
    with (
        tc.tile_pool(name="main", bufs=1) as pool,
        tc.tile_pool(name="vt_psum", bufs=3, space="PSUM") as vpp,
        tc.tile_pool(name="ut_psum", bufs=1, space="PSUM") as upp,
        tc.tile_pool(name="out_psum", bufs=4, space="PSUM") as opp,
    ):
        def t(shape, tag, dtype=F32):
            return pool.tile([128, *shape], dtype, name=tag, tag=tag)

        dve = nc.vector
        act = nc.scalar
        gps = nc.gpsimd

        # ---- input DMAs first ----
        params = t([G * 5], "params")
        nc.sync.dma_start(
            out=params[:, 0 : GP * 5],
            in_=pred.rearrange("(p j) c -> p (j c)", p=128),
        )
        act.dma_start(
            out=params[:, GP * 5 : G * 5],
            in_=targ.rearrange("(p j) c -> p (j c)", p=128),
        )

        P5 = params.rearrange("p (g c) -> p c g", c=5)
        kap = P5[:, 3, :]
        bet = P5[:, 4, :]
        kap_p, bet_p = kap[:, 0:GP], bet[:, 0:GP]
        kap_t, bet_t = kap[:, GP:G], bet[:, GP:G]

        # ---- constants ----
        half_pi = pool.tile([128, 1], F32, name="half_pi", tag="half_pi")
        gps.memset(half_pi, PI / 2)
        eps_c = pool.tile([128, 1], F32, name="eps_c", tag="eps_c")
        gps.memset(eps_c, EPS)
        ident = pool.tile([128, 128], BF16, name="ident", tag="ident")
        make_identity(nc, ident)

        # dummy sin: trig table loads during the input DMA
        dmy = pool.tile([128, 1], F32, name="dmy", tag="dmy")
        act.activation(dmy[:], half_pi[:], AF.Sin)

        # ---- workspace W: slot axis x 18 group columns.
        # 0 ce, 1 ca, 2 cp | 3 se, 4 sa, 5 sp | 6 g1x, 7 g1y, 8 g1z |
        # 9 m2, 10 m4 | 11 spce, 12 spse, 13 cpce, 14 cpse |
        # 15 m2ce, 16 m2se, 17 m4ce, 18 m4se |
        # 19 g2x, 20 g2y, 21 g2z | 22 G3x, 23 G3y, 24 G3z |
        # 25:31 squares [g2,G3] | 31:37 offdiags | 37:40 dVdiag,
        # 40:43 dVoff | 43 km, 44 kp, 45 LNIN, 46 lnprod |
        # 47:53 p1 (pred cols) / 48:52 VF f1-4 fp32 (target cols) |
        # 58:61 absv | 61 b2
        S = 62
        W = t([S, G], "W")

        U = t([20, GP], "U")

        def u(i):
            return U[:, i, :]

        # ---- Pool: kappa/beta shared (needs only the DMA) ----
        gps.tensor_add(W[:, 61, :], bet, bet)  # b2 = 2*beta
        gps.tensor_sub(W[:, 43, :], kap, W[:, 61, :])  # km
        gps.tensor_add(W[:, 44, :], kap, W[:, 61, :])  # kp
        gps.tensor_mul(W[:, 45, :], W[:, 43, :], W[:, 44, :])  # LNIN

        # ---- DVE: abs + U-chain head (only needs the DMA) ----
        angles = P5[:, 0:3, :]
        absv = W[:, 58:61, :]
        dve.scalar_tensor_tensor(absv, angles, -1.0, angles, ALU.mult, ALU.max)
        dve.tensor_mul(u(0), kap_p, kap_p)  # x2
        dve.scalar_tensor_tensor(u(1), bet_p, 4.0, bet_p, ALU.mult, ALU.mult)  # s
        dve.tensor_sub(u(2), u(0), u(1))  # D
        dve.reciprocal(u(3), u(2))  # rec

        # ---- ACT: trig + ln (one ln-table load in between) ----
        act.activation(W[:, 3:6, :], angles, AF.Sin)  # se, sa, sp
        act.activation(W[:, 0:3, :], absv, AF.Sin, bias=half_pi, scale=-1.0)
        act.activation(W[:, 46, :], W[:, 45, :], AF.Ln, bias=eps_c)  # lnprod

        def rep_outer(ap, n):
            return ap.unsqueeze(2).broadcast_to([128, ap.shape[1], n, ap.shape[2]])

        def rep_inner(ap, n):
            return ap.unsqueeze(1).broadcast_to([128, n, ap.shape[1], ap.shape[2]])

        # ---- DVE: gamma chain ----
        cpsp = W[:, 2:6:3, :]
        cese = W[:, 0:4:3, :]
        sa_b = W[:, 4:5, :].broadcast_to([128, 2, G])
        dve.tensor_mul(W[:, 9:11, :], cpsp, W[:, 1:2, :].broadcast_to([128, 2, G]))
        dve.tensor_copy(W[:, 6:7, :], W[:, 1:2, :])
        dve.tensor_mul(W[:, 7:9, :], sa_b, cese)
        aux1_out = W[:, 11:15, :].rearrange("p (a b) g -> p a b g", a=2)
        dve.tensor_mul(aux1_out, rep_outer(W[:, 5:1:-3, :], 2), rep_inner(cese, 2))
        aux2_out = W[:, 15:19, :].rearrange("p (a b) g -> p a b g", a=2)
        dve.tensor_mul(aux2_out, rep_outer(W[:, 9:11, :], 2), rep_inner(cese, 2))
        dve.scalar_tensor_tensor(W[:, 19:23:3, :], cpsp, -1.0, sa_b, ALU.mult, ALU.mult)
        dve.tensor_sub(W[:, 20:25:4, :], W[:, 15:19:3, :], W[:, 12:14, :])
        dve.tensor_add(W[:, 21:24:2, :], W[:, 16:18, :], W[:, 11:15:3, :])

        # ---- p1 products (Pool; slots 47:53, pred cols) + V pair products
        # (full 18 columns: contiguous APs beat GT-sliced 4D ones) ----
        g1p = W[:, 6:9, 0:GP]
        gps.tensor_mul(W[:, 47:50, 0:GP], g1p, g1p)
        gps.tensor_mul(
            W[:, 50:52, 0:GP],
            W[:, 6:7, 0:GP].broadcast_to([128, 2, GP]),
            W[:, 7:9, 0:GP],
        )
        gps.tensor_mul(W[:, 52, 0:GP], W[:, 7, 0:GP], W[:, 8, 0:GP])
        g6 = W[:, 19:25, :]
        dve.tensor_mul(W[:, 25:31, :], g6, g6)  # squares
        dve.tensor_mul(  # g2xy, g2xz
            W[:, 31:33, :], W[:, 19:20, :].broadcast_to([128, 2, G]), W[:, 20:22, :]
        )
        gps.tensor_mul(  # G3xy, G3xz
            W[:, 34:36, :], W[:, 22:23, :].broadcast_to([128, 2, G]), W[:, 23:25, :]
        )
        gps.tensor_mul(W[:, 33:39:3, :], W[:, 20:26:3, :], W[:, 21:27:3, :])  # yz

        # ---- DVE: U-chain middle + dV subs ----
        dve.tensor_sub(u(5), u(2), kap_p)  # n2 = D - k
        dve.tensor_mul(u(6), u(5), u(3))  # l1
        dve.scalar_tensor_tensor(u(7), kap_p, -1.0, u(0), ALU.add, ALU.mult)  # t2
        dve.scalar_tensor_tensor(u(8), kap_p, 0.5, u(1), ALU.add, ALU.mult)  # ks+s/2
        dve.tensor_sub(u(9), u(7), u(8))  # Qh
        dve.tensor_sub(W[:, 37:40, GP:G], W[:, 28:31, GP:G], W[:, 25:28, GP:G])
        dve.tensor_sub(W[:, 40:43, GP:G], W[:, 34:37, GP:G], W[:, 31:34, GP:G])
        dve.tensor_mul(u(10), u(3), u(3))  # rec^2
        dve.tensor_mul(u(11), u(9), u(10))  # l2
        dve.tensor_sub(u(12), u(6), u(11))  # dE
        dve.tensor_mul(u(13), kap_p, u(6))  # kadot

        # ---- V features: f1-f4 (c_b, -k*g_b1) get fp32 + bf16 hi/lo; the
        # six beta features go straight to bf16 (their lo would contribute
        # < 3e-3 rel).  V30 = [Vh(11); Vl(4); Vh(11); Vl(4)] via rep-2 outs.
        VH = t([K2, GT], "VH", BF16)
        UH = t([K2, GP], "UH", BF16)
        vh2 = VH.rearrange("p (r s) g -> p r s g", r=2)
        gps.memset(vh2[:, :, 0, :], 1.0)  # V feature 0 == 1 (exact in bf16)
        # two 8-group halves so the first transposes start ~0.5us earlier
        for h in range(2):
            gl, gr = GP + 8 * h, GP + 8 * h + 8
            vl, vr = 8 * h, 8 * h + 8
            ktb = kap_t[:, vl:vr].unsqueeze(1).broadcast_to([128, 3, 8])
            btb = bet_t[:, vl:vr].unsqueeze(1).broadcast_to([128, 3, 8])
            dve.scalar_tensor_tensor(
                W[:, 48, gl:gr], W[:, 46, gl:gr], -0.5, kap_t[:, vl:vr],
                ALU.mult, ALU.add,
            )
            dve.scalar_tensor_tensor(
                W[:, 49:52, gl:gr], W[:, 6:9, gl:gr], -1.0, ktb,
                ALU.mult, ALU.mult,
            )
            gps.tensor_mul(VH[:, 5:8, vl:vr], W[:, 37:40, gl:gr], btb)
            gps.tensor_mul(VH[:, KB + 5 : KB + 8, vl:vr], W[:, 37:40, gl:gr], btb)
            dve.scalar_tensor_tensor(
                VH[:, 8:11, vl:vr], W[:, 40:43, gl:gr], 2.0, btb,
                ALU.mult, ALU.mult,
            )
            dve.scalar_tensor_tensor(
                VH[:, KB + 8 : KB + 11, vl:vr], W[:, 40:43, gl:gr], 2.0, btb,
                ALU.mult, ALU.mult,
            )
            act.copy(
                vh2[:, :, 1:5, vl:vr], rep_inner(W[:, 48:52, gl:gr], 2)
            )  # hi f1-4
            dve.tensor_sub(  # lo f1-4
                vh2[:, :, K : K + KL, vl:vr],
                rep_inner(W[:, 48:52, gl:gr], 2),
                rep_inner(VH[:, 1:5, vl:vr], 2),
            )
        # A = 0.5*lnprod - k + k*l1
        dve.scalar_tensor_tensor(
            u(14), W[:, 46, 0:GP], 0.5, kap_p, ALU.mult, ALU.subtract
        )

        # ---- Pool: U features + split ----
        # U30 = [Uh(11); Uh(f1-4); Ul(11); Ul(f1-4)]
        UF = t([K, GP], "UF")
        gps.memset(UF[:, 1, :], 1.0)
        l1b = U[:, 6:7, :].broadcast_to([128, 3, GP])
        deb = U[:, 12:13, :].broadcast_to([128, 3, GP])
        de2 = U[:, 12:13, :].broadcast_to([128, 2, GP])
        gps.tensor_mul(UF[:, 2:5, :], g1p, l1b)
        gps.tensor_mul(UF[:, 5:8, :], W[:, 47:50, 0:GP], deb)
        gps.tensor_mul(UF[:, 8:10, :], W[:, 50:52, 0:GP], de2)
        gps.tensor_mul(UF[:, 10, :], W[:, 52, 0:GP], U[:, 12, :])
        gps.tensor_add(UF[:, 0, :], u(14), u(13))

        gps.tensor_copy(UH[:, 0:K, :], UF[:])  # hi
        gps.tensor_copy(UH[:, K:KB, :], UH[:, 1 : 1 + KL, :])  # hi dup f1-4
        gps.tensor_sub(UH[:, KB : KB + K, :], UF[:], UH[:, 0:K, :])  # lo
        gps.tensor_copy(UH[:, KB + K : K2, :], UH[:, KB + 1 : KB + 1 + KL, :])

        # ---- PE: V transposes q0..q3 then U transposes; psum->SBUF copies
        # chase on DVE (q0, q2, UT) / ACT (q1, q3) ----
        VT = pool.tile([K2, M], BF16, name="VT", tag="VT")
        utp = upp.tile([K2, 1024], BF16, name="utp", tag="utp")
        UT = pool.tile([K2, NS], BF16, name="UT", tag="UT")
        for q in range(4):
            vtp = vpp.tile([K2, 1024], BF16, name="vtp", tag="vtp")

            for jj in range(4):
                j = q * 4 + jj
                nc.tensor.transpose(
                    vtp[:, jj * 128 : (jj + 1) * 128], VH[:, :, j], ident[:]
                )
            if q == 3:
                # last chunk gates the first matmul: split across engines
                dve.tensor_copy(VT[:, 1536:1792], vtp[:, 0:256])
                act.copy(VT[:, 1792:2048], vtp[:, 256:512])
            elif q % 2 == 0:
                dve.tensor_copy(VT[:, q * 512 : (q + 1) * 512], vtp[:, 0:512])
            else:
                act.copy(VT[:, q * 512 : (q + 1) * 512], vtp[:, 0:512])
        # U transposes last (the Pool U-tail lands later than the V halves);
        # interleave copy on DVE so it overlaps the ACT-side q3 half-copy
        for j in range(GP):
            nc.tensor.transpose(utp[:, j * 128 : (j + 1) * 128], UH[:, :, j], ident[:])
        dve.tensor_copy(
            UT.rearrange("k (p j) -> k j p", j=GP),
            utp[:, 0 : GP * 128].rearrange("k (j p) -> k j p", p=128),
        )

        # ---- main matmuls (bf16, K=30): moving AP emits cols in m order.
        # PSUM->SBUF copies and HBM DMAs go in 256-col halves on both
        # engines/rings so the final block's drain is short.
        VTv = VT.rearrange("k (j p) -> k p j", p=128)  # col m = 16p + j
        outv = out.rearrange("(t p) m -> p t m", p=128)
        for c in range(4):
            for ti in range(GP):
                ops = opp.tile([128, 512], F32, name="ops", tag="ops")
                nc.tensor.matmul(
                    ops[:],
                    UT[:, 128 * ti : 128 * (ti + 1)],
                    VTv[:, 32 * c : 32 * (c + 1), :],
                    start=True,
                    stop=True,
                )
                out_sb = pool.tile(
                    [128, 512], F32, name="out_sb", tag="out_sb", bufs=6
                )
                # both halves on DVE: the ACT queue (copy+DMA issue) could
                # not keep the 860ns/block matmul pace and stalled the PSUM
                # recycle; ACT now only issues its ring's DMA
                last = c == 3 and ti == GP - 1
                if last:
                    # drain the final block in quarters on both engines/rings
                    dve.tensor_copy(out_sb[:, 0:128], ops[:, 0:128])
                    act.copy(out_sb[:, 128:256], ops[:, 128:256])
                    dve.tensor_copy(out_sb[:, 256:384], ops[:, 256:384])
                    act.copy(out_sb[:, 384:512], ops[:, 384:512])
                    for qq in range(4):
                        ring = nc.sync if qq % 2 == 0 else act
                        ring.dma_start(
                            out=outv[:, ti, 512 * c + 128 * qq : 512 * c + 128 * (qq + 1)],
                            in_=out_sb[:, 128 * qq : 128 * (qq + 1)],
                        )
                else:
                    dve.tensor_copy(out_sb[:, 0:256], ops[:, 0:256])
                    dve.tensor_copy(out_sb[:, 256:512], ops[:, 256:512])
                    nc.sync.dma_start(
                        out=outv[:, ti, 512 * c : 512 * c + 256],
                        in_=out_sb[:, 0:256],
                    )
                    act.dma_start(
                        out=outv[:, ti, 512 * c + 256 : 512 * (c + 1)],
                        in_=out_sb[:, 256:512],
                    )


def build():
    nc = bacc.Bacc()
    pred = nc.dram_tensor("pred", [NS, 5], F32, kind="ExternalInput")
    targ = nc.dram_tensor("targ", [M, 5], F32, kind="ExternalInput")
    out = nc.dram_tensor("out", [NS, M], F32, kind="ExternalOutput")
    with tile.TileContext(nc) as tc:
        _body(tc, pred[:], targ[:], out[:])
    nc.finalize()
    return nc


_NC_CACHE = None


def _get_nc():
    global _NC_CACHE
    if _NC_CACHE is None:
        _NC_CACHE = build()
    return _NC_CACHE


def kernel(kent_pred, kent_target, trace=False, tmpdir=None):
    from concourse.bass_utils import run_bass_kernel_spmd

    nc = _get_nc()
    kent_pred = np.ascontiguousarray(np.asarray(kent_pred, dtype=np.float32))
    kent_target = np.ascontiguousarray(np.asarray(kent_target, dtype=np.float32))
    in_maps = [
        {"pred": kent_pred[i * NS : (i + 1) * NS], "targ": kent_target}
        for i in range(NCORES)
    ]
    res = run_bass_kernel_spmd(
        nc, in_maps, core_ids=list(range(NCORES)), trace=trace, tmpdir=tmpdir
    )
    out = np.concatenate([r["out"] for r in res.results], axis=0)
    if trace:
        kernel.last_results = res
    return out

